# revision 12
# baseline (speedup 1.0000x reference)
"""Trainium2 Bass kernel for nn_Attn (Bahdanau-style attention scores).

Computation (per batch b of B=128):
    energy = tanh(enc[b] @ We.T + (hidden @ Wh.T)[b] + bias)   # (L, H)
    scores = energy @ v                                        # (L,)
    out[b] = softmax(scores)                                   # (1, L)

Sharding: batch data-parallel over 8 NeuronCores (16 batches/core);
weights replicated; no cross-device communication. Per core the
dominant matmul runs in the [h, l] orientation so the PE contracts
over d (=576):

    part_e[h, l] = sum_d WeT[d, h] * encT[d, l]      (lhsT=WeT, rhs=encT)

letting the host-computed c = hidden @ Wh.T + bias fuse into the tanh
as a per-partition activation bias, and the v-contraction run as a
z = sum_ht v_ht*en_ht DVE chain + one K=128 ones-matmul per l-half.

Design (v7; 235us baseline -> ~165us, all compared at matched device
clock via the ACTIVATE-duration clock reference):
- Stage-1 dtypes: MOVING enc tiles fp8 e3m4 (4-bit mantissa, 1 col/cyc
  like bf16, 4x less HBM than f32; end-to-end rel err 1.01e-2 vs the
  2e-2 gate, numpy-predicted and HW-confirmed), STATIONARY We fp16
  (negligible We quant error). Walrus forbids mixing 32-bit with
  non-32-bit matmul inputs, so f32r can't pair with either. Microbench
  (single run, clock-controlled): 16-bit/fp8 stationary paces 216ns/MM
  vs f32r's 227; an earlier session's "bf16 268ns" was clock drift.
  fp8e4 DoubleRow is the only sub-1-cyc/row mode and its quant error
  sims at 2.9e-2 (hi/lo rescue is slot-for-slot zero-sum vs f32r).
- d=576 contraction: 4 full K=128 k-tiles + the 64-dim tail as
  row-packed K=64 matmul PAIRS (tile_position row groups 0/64, two ht
  outputs into separate PSUM banks concurrently; pair spans ~312ns vs
  2x216 serial or 2x216 of zero-padding). Rows 64..127 of the tail
  k-tile hold a host-side duplicate of dims 512..575.
- c-block (hidden @ Wh.T + b) computed on the HOST (34 MFLOP numpy)
  and shipped as a [128, 4*16] bias table: removes 16 PE matmuls, 1MB
  of prologue DMA, and the walrus-scheduler stalls waiting for it.
- One coalesced DMA descriptor per batch (5KB partition lines): the
  per-k-tile version paced ~130GB/s in 1KB packets and stalled the PE.
  Batch 0 (+ its weights) split kt0/kt1/rest so the kt-major first
  group starts on the first ~0.25MB.
- Stage-2 (ones-matmul / v-matmuls) for group g is emitted after group
  g+1's stage-1 so the PE never waits on the DVE z-chain (~308ns/group
  otherwise). The last batch's exp reads its score PSUM directly
  (chunk-B softmax), everything else softmaxes as one 15-row chunk
  that overlaps batch 15's compute. No max-subtraction: |s| < 7 so
  f32 exp cannot overflow.
- PE warmup: 3 junk matmuls at engine wake-up. Starting real work
  ASAP beats longer warmups: the HAM clock-gate half-rates the first
  ~3.4us of activity regardless, and cold-but-early beats warm-but-
  late.

Remaining structure at ~165us: ~8us engine-boot head (fixed), ~131us
stage-1 (94% of the 122.9us/core PE roofline at 1 cyc/row; the gap is
the pairs' exposed LDWEIGHTS, which cannot overlap a preceding
full-array matmul), ~8us stage-2/warm inline, ~3us softmax tail,
~3us fixed teardown drains. HW exec time for identical code drifts
>15% with device clock state - normalize before comparing runs.
"""

import numpy as np

import concourse.bacc as bacc
import concourse.mybir as mybir
import concourse.tile as tile
from concourse import bass_utils
from concourse.mybir import ActivationFunctionType as AF
from concourse.mybir import AluOpType
from concourse import bass_isa

N_CORES = 8
B, L, H = 128, 1024, 512
ONEHOT = 64
DE = H + ONEHOT          # 576, true contraction dim of the big matmul
DK = 64                  # tail contraction dims (576 = 4*128 + 64)
BL = B // N_CORES        # 16 batches per core
F32 = mybir.dt.float32
F32R = mybir.dt.float32r
BF16 = mybir.dt.bfloat16
F16 = mybir.dt.float16
FP8E3 = mybir.dt.float8e3

NKT = 5                                  # 4 full d-tiles + packed tail
NHT = H // 128                           # 4 h-tiles
NLH = L // 512                           # 2 l-halves (N=512 per matmul)
BLA = BL - 1                             # softmax chunk A: batches 0..14
EW = NKT * L                             # 5120 cols in a combined enc tile


def build(reps: int = 1, dt1=FP8E3, dtw=F16, dt2=F32R, nw: int = 3):
    """Build + trace the per-core Bass program. Returns the compiled nc.

    dt1: stage-1 MOVING enc dtype. dtw: STATIONARY We dtype.
    dt2: stage-2 dtype (z tiles + ones column).
    nw: number of PE warmup matmuls (HAM clock-gate fuel).
    """
    nc = bacc.Bacc(
        "TRN2", target_bir_lowering=False, debug=False, num_devices=N_CORES
    )
    # enc[b, p, kt*L + l] = encT[b, kt*128 + p, l]  (kt4: p>=64 dups p-64)
    enc = nc.dram_tensor("enc", [BL, 128, EW], dt1, kind="ExternalInput").ap()
    wet = nc.dram_tensor("wet", [128, NKT * H], dtw, kind="ExternalInput").ap()
    # cb[p, ht*BL + b] = (hidden @ Wh.T + bias)[b, ht*128 + p], host-computed
    cb = nc.dram_tensor("cb", [128, NHT * BL], F32, kind="ExternalInput").ap()
    vcol = nc.dram_tensor("vcol", [128, NHT], F32, kind="ExternalInput").ap()
    ones = nc.dram_tensor("ones", [128, 1], dt2, kind="ExternalInput").ap()
    vcolr = nc.dram_tensor("vcolr", [128, NHT], dt2, kind="ExternalInput").ap()
    out = nc.dram_tensor("out", [BL, L], F32, kind="ExternalOutput").ap()

    with tile.TileContext(nc) as tc:
        with (
            tc.tile_pool(name="const", bufs=1) as cpool,
            tc.tile_pool(name="encp", bufs=6) as epool,
            tc.tile_pool(name="energy", bufs=8) as gpool,
            tc.tile_pool(name="soft", bufs=1) as spool,
            tc.tile_pool(name="stage", bufs=4) as stpool,
            tc.tile_pool(name="ps1", bufs=6, space="PSUM") as ps1,
            tc.tile_pool(name="ps3", bufs=2, space="PSUM") as ps3,
        ):
            # ---- PE warmup fuel: memset tile (no DMA dependency) so the
            # warmup starts at the Tensor engine's wake-up instead of
            # waiting out a DMA semaphore.
            wtile = cpool.tile([128, 512], BF16, tag="wtile", name="wtile")
            nc.vector.memset(wtile[:], 1.0)

            # ---- replicated constants + batch 0 enc, few big descriptors.
            # wet/enc0 split at kt0 so batch 0's kt-major group starts on
            # the first ~0.25MB instead of the full 1.3MB.
            wet0 = cpool.tile([128, H], dtw, tag="wet0", name="wet0")
            nc.sync.dma_start(wet0[:], wet[:, 0:H])
            e00 = epool.tile([128, L], dt1, tag="e00", name="e00", bufs=1)
            nc.sync.dma_start(e00[:], enc[0, :, 0:L])
            wet1 = cpool.tile([128, H], dtw, tag="wet1", name="wet1")
            nc.sync.dma_start(wet1[:], wet[:, H : 2 * H])
            e01 = epool.tile([128, L], dt1, tag="e01", name="e01", bufs=1)
            nc.sync.dma_start(e01[:], enc[0, :, L : 2 * L])
            wetR = cpool.tile([128, (NKT - 2) * H], dtw, tag="wetR", name="wetR")
            nc.sync.dma_start(wetR[:], wet[:, 2 * H :])
            e0R = epool.tile([128, EW - 2 * L], dt1, tag="e0R", name="e0R", bufs=1)
            nc.sync.dma_start(e0R[:], enc[0, :, 2 * L :])
            cb_sb = cpool.tile([128, NHT * BL], F32, tag="cb", name="cb_sb")
            nc.sync.dma_start(cb_sb[:], cb[:, :])
            vcol_sb = cpool.tile([128, NHT], F32, tag="vcol", name="vcol_sb")
            nc.sync.dma_start(vcol_sb[:], vcol[:, :])
            ones128 = cpool.tile([128, 1], dt2, tag="ones128", name="ones128")
            nc.sync.dma_start(ones128[:], ones[:, :])
            vcolr_sb = cpool.tile([128, NHT], dt2, tag="vcolr", name="vcolr_sb")
            nc.sync.dma_start(vcolr_sb[:], vcolr[:, :])

            def wslice(kt, ht, rows=slice(0, 128)):
                if kt == 0:
                    return wet0[rows, ht * 128 : (ht + 1) * 128]
                if kt == 1:
                    return wet1[rows, ht * 128 : (ht + 1) * 128]
                o = (kt - 2) * H
                return wetR[rows, o + ht * 128 : o + (ht + 1) * 128]

            def mk_eslice(et):
                if et is None:  # batch 0: split tiles
                    def f(kt, lh, rows=slice(0, 128)):
                        if kt == 0:
                            return e00[rows, lh * 512 : (lh + 1) * 512]
                        if kt == 1:
                            return e01[rows, lh * 512 : (lh + 1) * 512]
                        o = (kt - 2) * L
                        return e0R[rows, o + lh * 512 : o + (lh + 1) * 512]
                else:
                    def f(kt, lh, rows=slice(0, 128)):
                        o = kt * L
                        return et[rows, o + lh * 512 : o + (lh + 1) * 512]
                return f

            def emit_pair(es, lh, pes, p0, p1):
                """K=64 tail as a row-packed concurrent pair (rows 0/64)."""
                for ht, r0 in ((p0, 0), (p1, 64)):
                    rows = slice(r0, r0 + 64)
                    nc.tensor.matmul(
                        pes[ht][:], lhsT=wslice(4, ht, rows),
                        rhs=es(4, lh, rows),
                        start=False, stop=True, skip_group_check=True,
                    )

            def emit_group(et, lh, pes, kt_major=False):
                """Stage-1 matmuls for one (batch, l-half): 4 full k-tiles
                per ht plus the K=64 tail as two row-packed pairs."""
                es = mk_eslice(et)
                if kt_major:
                    for kt in range(4):
                        for ht in range(4):
                            nc.tensor.matmul(
                                pes[ht][:], lhsT=wslice(kt, ht),
                                rhs=es(kt, lh),
                                start=(kt == 0), stop=False,
                            )
                    emit_pair(es, lh, pes, 0, 1)
                    emit_pair(es, lh, pes, 2, 3)
                else:
                    for hp in range(2):
                        for ht in (2 * hp, 2 * hp + 1):
                            for kt in range(4):
                                nc.tensor.matmul(
                                    pes[ht][:], lhsT=wslice(kt, ht),
                                    rhs=es(kt, lh),
                                    start=(kt == 0), stop=False,
                                )
                        emit_pair(es, lh, pes, 2 * hp, 2 * hp + 1)

            for _rep in range(reps):
                # ---- PE warmup: junk matmuls bridging until enc0 lands.
                warm = ps1.tile([128, 512], F32, tag="ps1", name="warm")
                for w in range(nw):
                    nc.tensor.matmul(
                        warm[:], lhsT=wtile[:, 0:128], rhs=wtile[:],
                        start=(w == 0), stop=(w == nw - 1),
                    )

                # ---- batch 0, l-half 0: stage-1 BEFORE the c-block,
                # kt-major so the group rolls as soon as wet+enc0 land.
                pes00 = [
                    ps1.tile([128, 512], F32, tag="ps1", name=f"pe0_0_{ht}")
                    for ht in range(4)
                ]
                emit_group(None, 0, pes00, kt_major=True)

                # scores for batches 0..14 park in one SBUF tile whose
                # softmax overlaps batch 15's compute; batch 15's exp reads
                # its score PSUM tiles directly (no staging round-trip).
                scoresA = spool.tile([BLA, L], F32, tag="scoresA", name="scoresA")
                exB = spool.tile([1, L], F32, tag="exB", name="exB")
                smB = [spool.tile([1, 1], F32, tag=f"smB{i}", name=f"smB{i}")
                       for i in range(2)]

                # ---- main loop over local batches. The stage-2 PE work
                # (ones-matmul / v-matmuls) for group g is emitted AFTER
                # group g+1's stage-1 matmuls so the PE never waits on the
                # DVE z-chain (v4 trace: ~308ns LDW stall per group).
                pending = []
                expB_pending = []

                def emit_stage2(b, lh, ens, z):
                    if z is not None:
                        # scores[l] = sum_p z[p, l] on the (idle) GpSimd
                        # engine; the PE never touches stage-2 for these
                        red = stpool.tile([128, 512], F32, tag="red",
                                          name=f"red{b}_{lh}", bufs=2)
                        nc.gpsimd.partition_all_reduce(
                            red[:], z[:], channels=128,
                            reduce_op=bass_isa.ReduceOp.add,
                        )
                        nc.sync.dma_start(
                            scoresA[b : b + 1, lh * 512 : (lh + 1) * 512],
                            red[0:1, :],
                        )
                        return
                    else:
                        ps_s = ps3.tile(
                            [1, 512], F32, tag="pss", name=f"ps_s{b}_{lh}"
                        )
                        for ht in range(4):
                            nc.tensor.matmul(
                                ps_s[:],
                                lhsT=vcolr_sb[:, ht : ht + 1],
                                rhs=ens[ht][:],
                                start=(ht == 0), stop=(ht == 3),
                            )
                    if b < BLA:
                        # stage psum scores out and park them batch-major
                        st = stpool.tile([1, 512], F32, tag="st",
                                         name=f"st{b}_{lh}")
                        nc.vector.tensor_copy(st[:], ps_s[:])
                        nc.sync.dma_start(
                            scoresA[b : b + 1, lh * 512 : (lh + 1) * 512],
                            st[:],
                        )
                    else:
                        # batch 15: exp straight from the score PSUM tile,
                        # deferred past every tanh so the in-order ACT
                        # queue is never blocked waiting on the v-matmuls
                        expB_pending.append((lh, ps_s))

                for b in range(BL):
                    if b == 0:
                        et = None
                    else:
                        et = epool.tile([128, EW], dt1, tag="encb",
                                        name=f"encb_{b}")
                        nc.sync.dma_start(et[:], enc[b, :, :])

                    for lh in range(NLH):
                        if b == 0 and lh == 0:
                            pes = pes00
                        else:
                            pes = [
                                ps1.tile([128, 512], F32, tag="ps1",
                                         name=f"pe{b}_{lh}_{ht}")
                                for ht in range(4)
                            ]
                            emit_group(et, lh, pes)
                        while pending:
                            emit_stage2(*pending.pop(0))
                        ens = []
                        for ht in range(4):
                            en_t = gpool.tile(
                                [128, 512], dt2, tag="en", name=f"en{b}_{lh}_{ht}"
                            )
                            nc.scalar.activation(
                                en_t[:], pes[ht][:], AF.Tanh,
                                bias=cb_sb[:, ht * BL + b : ht * BL + b + 1],
                            )
                            ens.append(en_t)
                        z = None
                        if b < BLA:
                            # z[p, l] = sum_ht v_ht[p] * en_ht[p, l]  (DVE).
                            for ht in range(4):
                                zn = stpool.tile(
                                    [128, 512], F32, tag="z",
                                    name=f"z{b}_{lh}_{ht}", bufs=8,
                                )
                                if z is None:
                                    nc.vector.tensor_scalar_mul(
                                        zn[:], ens[ht][:], vcol_sb[:, ht : ht + 1]
                                    )
                                else:
                                    nc.vector.scalar_tensor_tensor(
                                        zn[:], ens[ht][:],
                                        vcol_sb[:, ht : ht + 1], z[:],
                                        AluOpType.mult, AluOpType.add,
                                    )
                                z = zn
                        pending.append((b, lh, ens, z))
                while pending:
                    emit_stage2(*pending.pop(0))

                # ---- softmax. No max-subtraction: scores = v . tanh(...)
                # are O(1) (|s| < 7), so f32 exp cannot overflow and the
                # result matches the max-shifted softmax to ~1 ulp.
                # Chunk A (batches 0..14) overlaps batch 15's compute.
                ex = spool.tile([BLA, L], F32, tag="exA", name="exA")
                sm = spool.tile([BLA, 1], F32, tag="smA", name="smA")
                nc.scalar.activation(ex[:], scoresA[:], AF.Exp, accum_out=sm[:])
                rc = spool.tile([BLA, 1], F32, tag="rcA", name="rcA")
                nc.vector.reciprocal(rc[:], sm[:])
                oo = spool.tile([BLA, L], F32, tag="ooA", name="ooA")
                nc.vector.tensor_scalar_mul(oo[:], ex[:], rc[:, 0:1])
                nc.sync.dma_start(out[0:BLA, :], oo[:])
                # Batch 15 tail: exp pieces straight from PSUM, then
                # combine the two half-row sums.
                for lh, ps_s in expB_pending:
                    nc.scalar.activation(
                        exB[0:1, lh * 512 : (lh + 1) * 512], ps_s[:],
                        AF.Exp, accum_out=smB[lh][:],
                    )
                smT = spool.tile([1, 1], F32, tag="smT", name="smT")
                nc.vector.tensor_tensor(
                    smT[:], smB[0][:], smB[1][:], AluOpType.add
                )
                rcB = spool.tile([1, 1], F32, tag="rcB", name="rcB")
                nc.vector.reciprocal(rcB[:], smT[:])
                ooB = spool.tile([1, L], F32, tag="ooB", name="ooB")
                nc.vector.tensor_scalar_mul(ooB[:], exB[:], rcB[:, 0:1])
                nc.sync.dma_start(out[BLA : BLA + 1, :], ooB[:])

    nc.compile()
    return nc


_cached_nc = None


def _prep_in_maps(hidden, encoder_outputs, W, b, v, np1=None, npw=np.float16,
                  np2=np.float32):
    import ml_dtypes

    if np1 is None:
        np1 = ml_dtypes.float8_e3m4
    hidden = np.ascontiguousarray(hidden, dtype=np.float32)
    W = np.ascontiguousarray(W, dtype=np.float32)
    b = np.ascontiguousarray(b, dtype=np.float32)
    v = np.ascontiguousarray(v, dtype=np.float32)
    # (L, B, D) -> (B, D, L); combined per-batch tile [128, 5*L] where
    # col-block kt holds d-rows kt*128..+128 (kt4 rows 64.. duplicate
    # d=512..576 for the row-packed tail pairs).
    e = np.asarray(encoder_outputs, dtype=np.float32)
    encT = e.transpose(1, 2, 0).astype(np1)             # (B, DE, L)
    encC = np.empty((B, 128, NKT, L), dtype=np1)
    for kt in range(4):
        encC[:, :, kt, :] = encT[:, kt * 128 : (kt + 1) * 128, :]
    encC[:, 0:64, 4, :] = encT[:, 512:576, :]
    encC[:, 64:128, 4, :] = encT[:, 512:576, :]
    encC = encC.reshape(B, 128, EW)

    wetT = W[:, H:].T.astype(npw)                       # (DE, H)
    wetC = np.empty((128, NKT, H), dtype=npw)
    for kt in range(4):
        wetC[:, kt, :] = wetT[kt * 128 : (kt + 1) * 128, :]
    wetC[0:64, 4, :] = wetT[512:576, :]
    wetC[64:128, 4, :] = wetT[512:576, :]
    wetC = wetC.reshape(128, NKT * H)

    # c[b, h] = hidden @ Wh.T + bias (34 MFLOP, host-side)
    call = hidden @ W[:, :H].T + b                      # (B, H)
    vcol = np.ascontiguousarray(v.reshape(NHT, 128).T)  # (128, 4), f32
    ones = np.ones((128, 1), dtype=np2)
    in_maps = []
    for c in range(N_CORES):
        sl = slice(c * BL, (c + 1) * BL)
        # cb[p, ht*BL + b] = c[b, ht*128 + p]
        cbC = np.ascontiguousarray(
            call[sl].T.reshape(NHT, 128, BL).transpose(1, 0, 2)
            .reshape(128, NHT * BL), dtype=np.float32)
        in_maps.append(
            {
                "enc": encC[sl],
                "wet": wetC,
                "cb": cbC,
                "vcol": vcol,
                "ones": ones,
                "vcolr": vcol.astype(np2),
            }
        )
    return in_maps


def kernel(hidden, encoder_outputs, W, b, v):
    global _cached_nc
    if _cached_nc is None:
        _cached_nc = build(reps=1)
    in_maps = _prep_in_maps(hidden, encoder_outputs, W, b, v)
    res = bass_utils.run_bass_kernel_spmd(
        _cached_nc, in_maps, core_ids=list(range(N_CORES))
    )
    outs = np.concatenate([res.results[c]["out"] for c in range(N_CORES)], axis=0)
    return outs[:, None, :].astype(np.float32)


# revision 13
# speedup vs baseline: 1.0751x; 1.0751x over previous
"""Trainium2 Bass kernel for nn_Attn (Bahdanau-style attention scores).

Computation (per batch b of B=128):
    energy = tanh(enc[b] @ We.T + (hidden @ Wh.T)[b] + bias)   # (L, H)
    scores = energy @ v                                        # (L,)
    out[b] = softmax(scores)                                   # (1, L)

Sharding: batch data-parallel over 8 NeuronCores (16 batches/core);
weights replicated; no cross-device communication. Per core the
dominant matmul runs in the [h, l] orientation so the PE contracts
over d (=576):

    part_e[h, l] = sum_d WeT[d, h] * encT[d, l]      (lhsT=WeT, rhs=encT)

letting the host-computed c = hidden @ Wh.T + bias fuse into the tanh
as a per-partition activation bias, and the v-contraction run as a
z = sum_ht v_ht*en_ht DVE chain + one K=128 ones-matmul per l-half.

Design (v7; 235us baseline -> ~165us, all compared at matched device
clock via the ACTIVATE-duration clock reference):
- Stage-1 dtypes: MOVING enc tiles fp8 e3m4 (4-bit mantissa, 1 col/cyc
  like bf16, 4x less HBM than f32; end-to-end rel err 1.01e-2 vs the
  2e-2 gate, numpy-predicted and HW-confirmed), STATIONARY We fp16
  (negligible We quant error). Walrus forbids mixing 32-bit with
  non-32-bit matmul inputs, so f32r can't pair with either. Microbench
  (single run, clock-controlled): 16-bit/fp8 stationary paces 216ns/MM
  vs f32r's 227; an earlier session's "bf16 268ns" was clock drift.
  fp8e4 DoubleRow is the only sub-1-cyc/row mode and its quant error
  sims at 2.9e-2 (hi/lo rescue is slot-for-slot zero-sum vs f32r).
- d=576 contraction: 4 full K=128 k-tiles + the 64-dim tail as
  row-packed K=64 matmul PAIRS (tile_position row groups 0/64, two ht
  outputs into separate PSUM banks concurrently; pair spans ~312ns vs
  2x216 serial or 2x216 of zero-padding). Rows 64..127 of the tail
  k-tile hold a host-side duplicate of dims 512..575.
- c-block (hidden @ Wh.T + b) computed on the HOST (34 MFLOP numpy)
  and shipped as a [128, 4*16] bias table: removes 16 PE matmuls, 1MB
  of prologue DMA, and the walrus-scheduler stalls waiting for it.
- One coalesced DMA descriptor per batch (5KB partition lines): the
  per-k-tile version paced ~130GB/s in 1KB packets and stalled the PE.
  Batch 0 (+ its weights) split kt0/kt1/rest so the kt-major first
  group starts on the first ~0.25MB.
- Stage-2 (ones-matmul / v-matmuls) for group g is emitted after group
  g+1's stage-1 so the PE never waits on the DVE z-chain (~308ns/group
  otherwise). The last batch's exp reads its score PSUM directly
  (chunk-B softmax), everything else softmaxes as one 15-row chunk
  that overlaps batch 15's compute. No max-subtraction: |s| < 7 so
  f32 exp cannot overflow.
- PE warmup: 3 junk matmuls at engine wake-up. Starting real work
  ASAP beats longer warmups: the HAM clock-gate half-rates the first
  ~3.4us of activity regardless, and cold-but-early beats warm-but-
  late.

Remaining structure at ~165us: ~8us engine-boot head (fixed), ~131us
stage-1 (94% of the 122.9us/core PE roofline at 1 cyc/row; the gap is
the pairs' exposed LDWEIGHTS, which cannot overlap a preceding
full-array matmul), ~8us stage-2/warm inline, ~3us softmax tail,
~3us fixed teardown drains. HW exec time for identical code drifts
>15% with device clock state - normalize before comparing runs.
"""

import numpy as np

import concourse.bacc as bacc
import concourse.mybir as mybir
import concourse.tile as tile
from concourse import bass_utils
from concourse.mybir import ActivationFunctionType as AF
from concourse.mybir import AluOpType

N_CORES = 8
B, L, H = 128, 1024, 512
ONEHOT = 64
DE = H + ONEHOT          # 576, true contraction dim of the big matmul
DK = 64                  # tail contraction dims (576 = 4*128 + 64)
BL = B // N_CORES        # 16 batches per core
F32 = mybir.dt.float32
F32R = mybir.dt.float32r
BF16 = mybir.dt.bfloat16
F16 = mybir.dt.float16
FP8E3 = mybir.dt.float8e3

NKT = 5                                  # 4 full d-tiles + packed tail
NHT = H // 128                           # 4 h-tiles
NLH = L // 512                           # 2 l-halves (N=512 per matmul)
BLA = BL - 1                             # softmax chunk A: batches 0..14
EW = NKT * L                             # 5120 cols in a combined enc tile


def build(reps: int = 1, dt1=FP8E3, dtw=F16, dt2=F32R, nw: int = 3):
    """Build + trace the per-core Bass program. Returns the compiled nc.

    dt1: stage-1 MOVING enc dtype. dtw: STATIONARY We dtype.
    dt2: stage-2 dtype (z tiles + ones column).
    nw: number of PE warmup matmuls (HAM clock-gate fuel).
    """
    nc = bacc.Bacc(
        "TRN2", target_bir_lowering=False, debug=False, num_devices=N_CORES
    )
    # enc[b, p, kt*L + l] = encT[b, kt*128 + p, l]  (kt4: p>=64 dups p-64)
    enc = nc.dram_tensor("enc", [BL, 128, EW], dt1, kind="ExternalInput").ap()
    wet = nc.dram_tensor("wet", [128, NKT * H], dtw, kind="ExternalInput").ap()
    # cb[p, ht*BL + b] = (hidden @ Wh.T + bias)[b, ht*128 + p], host-computed
    cb = nc.dram_tensor("cb", [128, NHT * BL], F32, kind="ExternalInput").ap()
    vcol = nc.dram_tensor("vcol", [128, NHT], F32, kind="ExternalInput").ap()
    ones = nc.dram_tensor("ones", [128, 1], dt2, kind="ExternalInput").ap()
    vcolr = nc.dram_tensor("vcolr", [128, NHT], dt2, kind="ExternalInput").ap()
    out = nc.dram_tensor("out", [BL, L], F32, kind="ExternalOutput").ap()

    with tile.TileContext(nc) as tc:
        with (
            tc.tile_pool(name="const", bufs=1) as cpool,
            tc.tile_pool(name="encp", bufs=6) as epool,
            tc.tile_pool(name="energy", bufs=8) as gpool,
            tc.tile_pool(name="soft", bufs=1) as spool,
            tc.tile_pool(name="stage", bufs=4) as stpool,
            tc.tile_pool(name="ps1", bufs=6, space="PSUM") as ps1,
            tc.tile_pool(name="ps3", bufs=2, space="PSUM") as ps3,
        ):
            # ---- PE warmup fuel: memset tile (no DMA dependency) so the
            # warmup starts at the Tensor engine's wake-up instead of
            # waiting out a DMA semaphore.
            wtile = cpool.tile([128, 512], BF16, tag="wtile", name="wtile")
            nc.vector.memset(wtile[:], 1.0)

            # ---- replicated constants + batch 0 enc, few big descriptors.
            # wet/enc0 split at kt0 so batch 0's kt-major group starts on
            # the first ~0.25MB instead of the full 1.3MB.
            wet0 = cpool.tile([128, H], dtw, tag="wet0", name="wet0")
            nc.sync.dma_start(wet0[:], wet[:, 0:H])
            e00 = epool.tile([128, L], dt1, tag="e00", name="e00", bufs=1)
            nc.sync.dma_start(e00[:], enc[0, :, 0:L])
            wet1 = cpool.tile([128, H], dtw, tag="wet1", name="wet1")
            nc.sync.dma_start(wet1[:], wet[:, H : 2 * H])
            e01 = epool.tile([128, L], dt1, tag="e01", name="e01", bufs=1)
            nc.sync.dma_start(e01[:], enc[0, :, L : 2 * L])
            wetR = cpool.tile([128, (NKT - 2) * H], dtw, tag="wetR", name="wetR")
            nc.sync.dma_start(wetR[:], wet[:, 2 * H :])
            e0R = epool.tile([128, EW - 2 * L], dt1, tag="e0R", name="e0R", bufs=1)
            nc.sync.dma_start(e0R[:], enc[0, :, 2 * L :])
            cb_sb = cpool.tile([128, NHT * BL], F32, tag="cb", name="cb_sb")
            nc.sync.dma_start(cb_sb[:], cb[:, :])
            vcol_sb = cpool.tile([128, NHT], F32, tag="vcol", name="vcol_sb")
            nc.sync.dma_start(vcol_sb[:], vcol[:, :])
            ones128 = cpool.tile([128, 1], dt2, tag="ones128", name="ones128")
            nc.sync.dma_start(ones128[:], ones[:, :])
            vcolr_sb = cpool.tile([128, NHT], dt2, tag="vcolr", name="vcolr_sb")
            nc.sync.dma_start(vcolr_sb[:], vcolr[:, :])

            def wslice(kt, ht, rows=slice(0, 128)):
                if kt == 0:
                    return wet0[rows, ht * 128 : (ht + 1) * 128]
                if kt == 1:
                    return wet1[rows, ht * 128 : (ht + 1) * 128]
                o = (kt - 2) * H
                return wetR[rows, o + ht * 128 : o + (ht + 1) * 128]

            def mk_eslice(et):
                if et is None:  # batch 0: split tiles
                    def f(kt, lh, rows=slice(0, 128)):
                        if kt == 0:
                            return e00[rows, lh * 512 : (lh + 1) * 512]
                        if kt == 1:
                            return e01[rows, lh * 512 : (lh + 1) * 512]
                        o = (kt - 2) * L
                        return e0R[rows, o + lh * 512 : o + (lh + 1) * 512]
                else:
                    def f(kt, lh, rows=slice(0, 128)):
                        o = kt * L
                        return et[rows, o + lh * 512 : o + (lh + 1) * 512]
                return f

            def emit_pair(es, lh, pes, p0, p1):
                """K=64 tail as a row-packed concurrent pair (rows 0/64)."""
                for ht, r0 in ((p0, 0), (p1, 64)):
                    rows = slice(r0, r0 + 64)
                    nc.tensor.matmul(
                        pes[ht][:], lhsT=wslice(4, ht, rows),
                        rhs=es(4, lh, rows),
                        start=False, stop=True, skip_group_check=True,
                    )

            def emit_group(et, lh, pes, kt_major=False):
                """Stage-1 matmuls for one (batch, l-half): 4 full k-tiles
                per ht plus the K=64 tail as two row-packed pairs."""
                es = mk_eslice(et)
                if kt_major:
                    for kt in range(4):
                        for ht in range(4):
                            nc.tensor.matmul(
                                pes[ht][:], lhsT=wslice(kt, ht),
                                rhs=es(kt, lh),
                                start=(kt == 0), stop=False,
                            )
                    emit_pair(es, lh, pes, 0, 1)
                    emit_pair(es, lh, pes, 2, 3)
                else:
                    for hp in range(2):
                        for ht in (2 * hp, 2 * hp + 1):
                            for kt in range(4):
                                nc.tensor.matmul(
                                    pes[ht][:], lhsT=wslice(kt, ht),
                                    rhs=es(kt, lh),
                                    start=(kt == 0), stop=False,
                                )
                        emit_pair(es, lh, pes, 2 * hp, 2 * hp + 1)

            for _rep in range(reps):
                # ---- PE warmup: junk matmuls bridging until enc0 lands.
                warm = ps1.tile([128, 512], F32, tag="ps1", name="warm")
                for w in range(nw):
                    nc.tensor.matmul(
                        warm[:], lhsT=wtile[:, 0:128], rhs=wtile[:],
                        start=(w == 0), stop=(w == nw - 1),
                    )

                # ---- batch 0, l-half 0: stage-1 BEFORE the c-block,
                # kt-major so the group rolls as soon as wet+enc0 land.
                pes00 = [
                    ps1.tile([128, 512], F32, tag="ps1", name=f"pe0_0_{ht}")
                    for ht in range(4)
                ]
                emit_group(None, 0, pes00, kt_major=True)

                # scores for batches 0..14 park in one SBUF tile whose
                # softmax overlaps batch 15's compute; batch 15's exp reads
                # its score PSUM tiles directly (no staging round-trip).
                scoresA = spool.tile([BLA, L], F32, tag="scoresA", name="scoresA")
                exB = spool.tile([1, L], F32, tag="exB", name="exB")
                smB = [spool.tile([1, 1], F32, tag=f"smB{i}", name=f"smB{i}")
                       for i in range(2)]

                # ---- main loop over local batches. The stage-2 PE work
                # (ones-matmul / v-matmuls) for group g is emitted AFTER
                # group g+1's stage-1 matmuls so the PE never waits on the
                # DVE z-chain (v4 trace: ~308ns LDW stall per group).
                pending = []
                expB_pending = []

                def emit_stage2(b, lh, ens, z):
                    ps_s = ps3.tile(
                        [1, 512], F32, tag="pss", name=f"ps_s{b}_{lh}"
                    )
                    if z is not None:
                        # scores[l] = sum_p z[p, l]: one K=128 ones-matmul
                        nc.tensor.matmul(
                            ps_s[:], lhsT=ones128[:], rhs=z[:],
                            start=True, stop=True,
                        )
                    else:
                        for ht in range(4):
                            nc.tensor.matmul(
                                ps_s[:],
                                lhsT=vcolr_sb[:, ht : ht + 1],
                                rhs=ens[ht][:],
                                start=(ht == 0), stop=(ht == 3),
                            )
                    if b < BLA:
                        # stage psum scores out and park them batch-major
                        st = stpool.tile([1, 512], F32, tag="st",
                                         name=f"st{b}_{lh}")
                        nc.vector.tensor_copy(st[:], ps_s[:])
                        nc.sync.dma_start(
                            scoresA[b : b + 1, lh * 512 : (lh + 1) * 512],
                            st[:],
                        )
                    else:
                        # batch 15: exp straight from the score PSUM tile,
                        # deferred past every tanh so the in-order ACT
                        # queue is never blocked waiting on the v-matmuls
                        expB_pending.append((lh, ps_s))

                for b in range(BL):
                    if b == 0:
                        et = None
                    else:
                        et = epool.tile([128, EW], dt1, tag="encb",
                                        name=f"encb_{b}")
                        nc.sync.dma_start(et[:], enc[b, :, :])

                    for lh in range(NLH):
                        if b == 0 and lh == 0:
                            pes = pes00
                        else:
                            pes = [
                                ps1.tile([128, 512], F32, tag="ps1",
                                         name=f"pe{b}_{lh}_{ht}")
                                for ht in range(4)
                            ]
                            emit_group(et, lh, pes)
                        while pending:
                            emit_stage2(*pending.pop(0))
                        ens = []
                        for ht in range(4):
                            en_t = gpool.tile(
                                [128, 512], dt2, tag="en", name=f"en{b}_{lh}_{ht}"
                            )
                            nc.scalar.activation(
                                en_t[:], pes[ht][:], AF.Tanh,
                                bias=cb_sb[:, ht * BL + b : ht * BL + b + 1],
                            )
                            ens.append(en_t)
                        z = None
                        if b < BLA:
                            # z[p, l] = sum_ht v_ht[p] * en_ht[p, l]  (DVE).
                            for ht in range(4):
                                zn = stpool.tile(
                                    [128, 512], dt2 if ht == 3 else F32, tag="z",
                                    name=f"z{b}_{lh}_{ht}", bufs=8,
                                )
                                if z is None:
                                    nc.vector.tensor_scalar_mul(
                                        zn[:], ens[ht][:], vcol_sb[:, ht : ht + 1]
                                    )
                                else:
                                    nc.vector.scalar_tensor_tensor(
                                        zn[:], ens[ht][:],
                                        vcol_sb[:, ht : ht + 1], z[:],
                                        AluOpType.mult, AluOpType.add,
                                    )
                                z = zn
                        pending.append((b, lh, ens, z))
                while pending:
                    emit_stage2(*pending.pop(0))

                # ---- softmax. No max-subtraction: scores = v . tanh(...)
                # are O(1) (|s| < 7), so f32 exp cannot overflow and the
                # result matches the max-shifted softmax to ~1 ulp.
                # Chunk A (batches 0..14) overlaps batch 15's compute.
                ex = spool.tile([BLA, L], F32, tag="exA", name="exA")
                sm = spool.tile([BLA, 1], F32, tag="smA", name="smA")
                nc.scalar.activation(ex[:], scoresA[:], AF.Exp, accum_out=sm[:])
                rc = spool.tile([BLA, 1], F32, tag="rcA", name="rcA")
                nc.vector.reciprocal(rc[:], sm[:])
                oo = spool.tile([BLA, L], F32, tag="ooA", name="ooA")
                nc.vector.tensor_scalar_mul(oo[:], ex[:], rc[:, 0:1])
                nc.sync.dma_start(out[0:BLA, :], oo[:])
                # Batch 15 tail: exp pieces straight from PSUM, then
                # combine the two half-row sums.
                for lh, ps_s in expB_pending:
                    nc.scalar.activation(
                        exB[0:1, lh * 512 : (lh + 1) * 512], ps_s[:],
                        AF.Exp, accum_out=smB[lh][:],
                    )
                smT = spool.tile([1, 1], F32, tag="smT", name="smT")
                nc.vector.tensor_tensor(
                    smT[:], smB[0][:], smB[1][:], AluOpType.add
                )
                rcB = spool.tile([1, 1], F32, tag="rcB", name="rcB")
                nc.vector.reciprocal(rcB[:], smT[:])
                ooB = spool.tile([1, L], F32, tag="ooB", name="ooB")
                nc.vector.tensor_scalar_mul(ooB[:], exB[:], rcB[:, 0:1])
                nc.sync.dma_start(out[BLA : BLA + 1, :], ooB[:])

    nc.compile()
    return nc


_cached_nc = None


def _prep_in_maps(hidden, encoder_outputs, W, b, v, np1=None, npw=np.float16,
                  np2=np.float32):
    import ml_dtypes

    if np1 is None:
        np1 = ml_dtypes.float8_e3m4
    hidden = np.ascontiguousarray(hidden, dtype=np.float32)
    W = np.ascontiguousarray(W, dtype=np.float32)
    b = np.ascontiguousarray(b, dtype=np.float32)
    v = np.ascontiguousarray(v, dtype=np.float32)
    # (L, B, D) -> (B, D, L); combined per-batch tile [128, 5*L] where
    # col-block kt holds d-rows kt*128..+128 (kt4 rows 64.. duplicate
    # d=512..576 for the row-packed tail pairs).
    e = np.asarray(encoder_outputs, dtype=np.float32)
    encT = e.transpose(1, 2, 0).astype(np1)             # (B, DE, L)
    encC = np.empty((B, 128, NKT, L), dtype=np1)
    for kt in range(4):
        encC[:, :, kt, :] = encT[:, kt * 128 : (kt + 1) * 128, :]
    encC[:, 0:64, 4, :] = encT[:, 512:576, :]
    encC[:, 64:128, 4, :] = encT[:, 512:576, :]
    encC = encC.reshape(B, 128, EW)

    wetT = W[:, H:].T.astype(npw)                       # (DE, H)
    wetC = np.empty((128, NKT, H), dtype=npw)
    for kt in range(4):
        wetC[:, kt, :] = wetT[kt * 128 : (kt + 1) * 128, :]
    wetC[0:64, 4, :] = wetT[512:576, :]
    wetC[64:128, 4, :] = wetT[512:576, :]
    wetC = wetC.reshape(128, NKT * H)

    # c[b, h] = hidden @ Wh.T + bias (34 MFLOP, host-side)
    call = hidden @ W[:, :H].T + b                      # (B, H)
    vcol = np.ascontiguousarray(v.reshape(NHT, 128).T)  # (128, 4), f32
    ones = np.ones((128, 1), dtype=np2)
    in_maps = []
    for c in range(N_CORES):
        sl = slice(c * BL, (c + 1) * BL)
        # cb[p, ht*BL + b] = c[b, ht*128 + p]
        cbC = np.ascontiguousarray(
            call[sl].T.reshape(NHT, 128, BL).transpose(1, 0, 2)
            .reshape(128, NHT * BL), dtype=np.float32)
        in_maps.append(
            {
                "enc": encC[sl],
                "wet": wetC,
                "cb": cbC,
                "vcol": vcol,
                "ones": ones,
                "vcolr": vcol.astype(np2),
            }
        )
    return in_maps


def kernel(hidden, encoder_outputs, W, b, v):
    global _cached_nc
    if _cached_nc is None:
        _cached_nc = build(reps=1)
    in_maps = _prep_in_maps(hidden, encoder_outputs, W, b, v)
    res = bass_utils.run_bass_kernel_spmd(
        _cached_nc, in_maps, core_ids=list(range(N_CORES))
    )
    outs = np.concatenate([res.results[c]["out"] for c in range(N_CORES)], axis=0)
    return outs[:, None, :].astype(np.float32)
